# revision 22
# baseline (speedup 1.0000x reference)
"""DenseGeneralAqt inference kernel for Trainium2 (8 NeuronCores).

out = (x @ dequant_int8(qkernel)) * qscale,  x:(2,2048,1024) f32,
qkernel:(1024,4096) int8, qscale:(1,4096) f32 -> out:(2,2048,4096) f32.

Strategy v2: 4x2 (M x N) shard grid, TRANSPOSED compute: W is the PE
stationary operand and x^T the moving one, so PSUM partitions equal the
output-feature axis and the per-channel qscale becomes a per-partition
[128,1] scalar that fuses into the PSUM->SBUF drain on EITHER the vector
or the scalar (ACT) engine - no 1 MB scale broadcast, drains split
across two engines. Weight k-tiles 0-3 arrive as host-cast fp16 on the
scalar HWDGE queue (no on-device cast gating the first sweeps); k-tiles
4-7 ride the gpsimd ring as int8 and are dequantized to fp16 on the
vector engine well before they are consumed. x^T loads k-tile-major on
the sync queue to match the k-outer sweep order. Output stores (f32,
transposed [N,M] per core; host untransposes) alternate sync/scalar
queues. The last group runs bank-outer so only one drain+store trails
the final matmul.
"""

import numpy as np

P = 128
B, S, D, F = 2, 2048, 1024, 4096
N_CORES = 8
MSH, NSH = 4, 2                   # shard grid: 4 m-blocks x 2 n-blocks
M_FULL = B * S                    # 4096 rows
M_CORE = M_FULL // MSH            # 1024 rows per core
N_CORE = F // NSH                 # 2048 cols per core
WK = D // P                       # 8 k-tiles
NT_CNT = N_CORE // P              # 16 n-tiles of 128
MH = 2                            # m halves of 512 (one PSUM bank each)
MHW = M_CORE // MH                # 512
NG = 4                            # groups of 4 n-tiles -> 8 banks/group
NPG = NT_CNT // NG                # 4 n-tiles per group
WARM = 36                         # PE clock-ramp dummy matmuls (bridge to
                                  # first-data arrival; >=3.7us busy needed
                                  # for the HAM clock gate to fully open)

_CACHE: dict = {}


def _build():
    import concourse.tile as tile
    from concourse import bacc, mybir

    nc = bacc.Bacc("TRN2", target_bir_lowering=False, debug=False)

    xt_dram = nc.dram_tensor("xt", [D, M_CORE], mybir.dt.float16, kind="ExternalInput")
    wf_dram = nc.dram_tensor("wf", [D, N_CORE], mybir.dt.float16, kind="ExternalInput")
    qs_dram = nc.dram_tensor("qs", [1, N_CORE], mybir.dt.float32, kind="ExternalInput")
    o_dram = nc.dram_tensor("o", [N_CORE, M_CORE], mybir.dt.float32, kind="ExternalOutput")

    xt_view = xt_dram[:, :].rearrange("(kt kp) m -> kp kt m", kp=P)    # [128, 8, 1024]
    wf_view = wf_dram[:, :].rearrange("(kt kp) n -> kp kt n", kp=P)    # [128, 8, 2048]
    qs_view = qs_dram[:, :].rearrange("o (nt p) -> p (o nt)", p=P)     # [128, 16]

    g0w = NPG * P                                                      # 512 cols

    with tile.TileContext(nc) as tc:
        with (
            tc.tile_pool(name="wf", bufs=1) as wfp,
            tc.tile_pool(name="xh", bufs=1) as xhp,
            tc.tile_pool(name="qs", bufs=1) as qp,
            tc.tile_pool(name="o", bufs=16) as op,
            tc.tile_pool(name="ps", bufs=8, space="PSUM") as pp,
        ):
            wf_sb = wfp.tile([P, WK, N_CORE], mybir.dt.float16, name="wf", tag="wf")
            xh = xhp.tile([P, WK, M_CORE], mybir.dt.float16, name="xh", tag="xh")
            qs = qp.tile([P, NT_CNT], mybir.dt.float32, name="qs", tag="qs")

            # scalar (ACT) HWDGE queue: weight k-tiles, group-0 columns of
            # every k-tile first so the first sweep is gated by the
            # smallest possible transfers (the very first by 32 KB).
            nc.scalar.dma_start(wf_sb[:, 0, 0:2 * P], wf_view[:, 0, 0:2 * P])
            nc.scalar.dma_start(wf_sb[:, 0, 2 * P:g0w], wf_view[:, 0, 2 * P:g0w])
            for kt in range(1, WK):
                nc.scalar.dma_start(wf_sb[:, kt, 0:g0w], wf_view[:, kt, 0:g0w])
            for kt in range(WK):
                nc.scalar.dma_start(
                    wf_sb[:, kt, g0w:N_CORE], wf_view[:, kt, g0w:N_CORE]
                )

            # sync (SP) HWDGE queue: x^T k-major as simple 2D transfers
            # (3D APs cost multi-us descriptor generation), qscale after
            # the first two (needed only at the first drain ~t0+14us).
            nc.sync.dma_start(xh[:, 0, 0:MHW], xt_view[:, 0, 0:MHW])
            nc.sync.dma_start(xh[:, 0, MHW:M_CORE], xt_view[:, 0, MHW:M_CORE])
            nc.sync.dma_start(xh[:, 1:2, :], xt_view[:, 1:2, :])
            nc.sync.dma_start(qs[:], qs_view)
            for kt in range(2, WK):
                nc.sync.dma_start(xh[:, kt:kt + 1, :], xt_view[:, kt:kt + 1, :])

            # PE warm-up on zeros: releases the HAM clock gate and bridges
            # the gap until the first weight/x tiles land (~t0+1.5us).
            warm = wfp.tile([P, P], mybir.dt.float16, name="warm", tag="warm")
            nc.gpsimd.memset(warm[:], 0)
            warm_ps = pp.tile([P, MHW], mybir.dt.float32, name="warm_ps", tag="ps")
            for _ in range(WARM):
                nc.tensor.matmul(warm_ps[:, 0:P], warm[:], warm[:])

            def w_ap(kt, nt):
                return wf_sb[:, kt, nt * P:(nt + 1) * P]

            def drain_store(nt, mh, ps_tile, bi, store_q=None):
                ot = op.tile([P, MHW], mybir.dt.float32, name=f"o{nt}_{mh}", tag="o")
                sc = qs[:, nt:nt + 1]
                if bi % 2 == 0:
                    nc.vector.tensor_scalar_mul(ot[:], ps_tile[:], sc)
                    q = nc.sync
                else:
                    nc.scalar.activation(
                        ot[:], ps_tile[:], mybir.ActivationFunctionType.Copy,
                        scale=sc,
                    )
                    q = nc.scalar
                (store_q or q).dma_start(
                    o_dram[nt * P:(nt + 1) * P, mh * MHW:(mh + 1) * MHW], ot[:]
                )

            def mm(ps_tile, kt, nt, mh, first, last):
                nc.tensor.matmul(
                    ps_tile[:],
                    w_ap(kt, nt),
                    xh[:, kt, mh * MHW:(mh + 1) * MHW],
                    start=first,
                    stop=last,
                )

            for g in range(NG):
                combos = [
                    (g * NPG + ntl, mh) for ntl in range(NPG) for mh in range(MH)
                ]
                if g < NG - 1:
                    # k-outer: each k-tile sweeps all 8 banks as soon as it
                    # (and its weights) are resident.
                    ps = {
                        c: pp.tile([P, MHW], mybir.dt.float32,
                                   name=f"ps{g}_{c[0]}_{c[1]}", tag="ps")
                        for c in combos
                    }
                    for kt in range(WK):
                        for c in combos:
                            mm(ps[c], kt, c[0], c[1], kt == 0, kt == WK - 1)
                    for bi, c in enumerate(combos):
                        drain_store(c[0], c[1], ps[c], bi)
                else:
                    # Last group bank-outer: drains+stores overlap the
                    # remaining matmuls; only one drain+store trails. The
                    # final (nt, mh) accumulates in two [128, 256] quarter
                    # banks so the very last drain+store is half-size and
                    # starts ~0.9us earlier.
                    for bi, c in enumerate(combos[:-1]):
                        ps_t = pp.tile([P, MHW], mybir.dt.float32,
                                       name=f"ps{g}_{c[0]}_{c[1]}", tag="ps")
                        for kt in range(WK):
                            mm(ps_t, kt, c[0], c[1], kt == 0, kt == WK - 1)
                        # Stores via sync so descriptor generation never
                        # blocks the next ACT drain.
                        drain_store(c[0], c[1], ps_t, bi, store_q=nc.sync)
                    nt, mh = combos[-1]
                    hw = MHW // 2
                    for h in range(2):
                        ps_q = pp.tile([P, hw], mybir.dt.float32,
                                       name=f"psq{h}", tag="ps")
                        lo = mh * MHW + h * hw
                        for kt in range(WK):
                            nc.tensor.matmul(
                                ps_q[:], w_ap(kt, nt), xh[:, kt, lo:lo + hw],
                                start=kt == 0, stop=kt == WK - 1,
                            )
                        ot = op.tile([P, hw], mybir.dt.float32,
                                     name=f"of{h}", tag="o")
                        sc = qs[:, nt:nt + 1]
                        if h == 0:
                            nc.vector.tensor_scalar_mul(ot[:], ps_q[:], sc)
                        else:
                            nc.scalar.activation(
                                ot[:], ps_q[:],
                                mybir.ActivationFunctionType.Copy, scale=sc,
                            )
                        nc.sync.dma_start(
                            o_dram[nt * P:(nt + 1) * P, lo:lo + hw], ot[:]
                        )

    nc.compile()
    return nc


def _get_nc():
    if "nc" not in _CACHE:
        _CACHE["nc"] = _build()
    return _CACHE["nc"]


def _run(x, qkernel, qscale, trace=False):
    from concourse.bass_utils import run_bass_kernel_spmd

    x = np.asarray(x, dtype=np.float32).reshape(M_FULL, D)
    xt = np.ascontiguousarray(x.T).astype(np.float16)    # [D, M_FULL]
    w = np.asarray(qkernel)
    if w.dtype != np.int8:
        w = w.astype(np.int8)
    s = np.asarray(qscale, dtype=np.float32).reshape(1, F)

    wf_sh = [
        np.ascontiguousarray(w[:, nb * N_CORE:(nb + 1) * N_CORE]).astype(np.float16)
        for nb in range(NSH)
    ]
    in_maps = []
    for c in range(N_CORES):
        mb, nb = c % MSH, c // MSH
        in_maps.append({
            "xt": np.ascontiguousarray(xt[:, mb * M_CORE:(mb + 1) * M_CORE]),
            "wf": wf_sh[nb],
            "qs": np.ascontiguousarray(s[:, nb * N_CORE:(nb + 1) * N_CORE]),
        })
    res = run_bass_kernel_spmd(
        _get_nc(), in_maps, core_ids=list(range(N_CORES)), trace=trace
    )
    out = np.empty((M_FULL, F), dtype=np.float32)
    for c in range(N_CORES):
        mb, nb = c % MSH, c // MSH
        out[mb * M_CORE:(mb + 1) * M_CORE, nb * N_CORE:(nb + 1) * N_CORE] = \
            res.results[c]["o"].T
    return out.reshape(B, S, F), res


def kernel(x, qkernel, qscale):
    try:
        out, _ = _run(x, qkernel, qscale, trace=False)
    except Exception:
        # One retry for transient device-side failures.
        out, _ = _run(x, qkernel, qscale, trace=False)
    return out


def kernel_traced(x, qkernel, qscale):
    out, res = _run(x, qkernel, qscale, trace=True)
    return out, res
